# revision 7
# baseline (speedup 1.0000x reference)
"""Trainium2 Bass kernel for ViTDet-style attention with decomposed
relative-position bias.

Problem shapes (hardcoded):
  x: (4, 32, 32, 768) f32, Wqkv: (768, 2304), Wproj: (768, 768),
  bproj: (768,), rel_pos_h/w: (63, 64).
  12 heads, head_dim 64, S = 32*32 = 1024.

Sharding: 48 (batch, head) pairs -> 6 heads per core, all of one batch per
core-pair. Each core computes its heads' attention and TWO partial output
projections (chunks 0-1 of its heads' Wproj rows, then chunk 2); the host
sums the four partials per batch and adds bproj.

v2 schedule notes (vs v1):
  - persistent weights arrive via 8 large multi-level-AP DMAs (striped
    over all 16 DMA engines) split across the sync + scalar HWDGE queues,
    instead of 28 serialized ~600ns sync-queue issues.
  - v_sb per k-block is [ones(64) | v 6 heads(384)]; the av stationary is
    a strided-column AP {ones, head i}, so the v cast is one contiguous
    DVE copy instead of a slow strided scatter.
  - output projection is split: chunks 0-1 run interleaved inside head
    4's kb loop (PE slack under the exp-bound phase), chunk 2 after head
    5 -> short tail. Host sums out_a + out_b (+ pair core) + bproj.

Device algorithm per core (bf16 matmuls, fp32 PSUM accumulation):
  - qkT = Wqk^T @ x^T  (x^T supplied pre-transposed by host; k pre-scaled)
  - v   = x @ Wv       (natural layout)
  - rel-pos bias computed in band form: h-axis via table product + DRAM
    bounce band-shift gather; w-axis via windowed stationaries (two
    h-values packed per 64x64 matmul, diagonal blocks used).
  - scoresT (k x q) = kaugT^T @ qaugT in ONE K=128 matmul per tile:
    aug rows 0-63 = kT / qT, 64-95 = one-hot(h) / BhT, 96-127 = one-hot(w)/BwT
    => rel-pos bias folded into the QK matmul for free.
  - eT = exp(scoresT) on ScalarE (no max subtraction; scores are O(1)).
  - avT (128 x q): rows 0-63 = denominator, rows 64-127 = out accum.
  - normalize: DVE reciprocal of av[0:64] + DVE multiply. No DMA bounce.
"""

import numpy as np

import concourse.bass as bass
import concourse.bacc as bacc
import concourse.mybir as mybir
import concourse.tile as tile
from concourse.bass_utils import run_bass_kernel_spmd

F32 = mybir.dt.float32
BF16 = mybir.dt.bfloat16

NH = 12          # total heads
C = 768
HD = 64
H = W = 32
S = H * W        # 1024
B = 4
NCORES = 8
HPC = NH * B // NCORES   # heads per core = 6
NCH = 6                  # C // 128 input-channel chunks
NKB = S // 128           # 8 k blocks
NQB = S // 128           # 8 q blocks
NHALF = 512              # matmul moving-dim half
VSEG = HPC * 2 * HD      # 768: [ones|v_i] x 6 heads per k block


def build_program():
    nc = bacc.Bacc("TRN2", target_bir_lowering=False, debug=False)

    xT = nc.declare_dram_parameter("xT", [C, S], BF16, isOutput=False)
    wqk = nc.declare_dram_parameter("wqk", [C, 2 * HPC * HD], BF16, isOutput=False)
    wv = nc.declare_dram_parameter("wv", [C, HPC * HD], BF16, isOutput=False)
    wproj = nc.declare_dram_parameter("wproj", [HPC * HD, C], BF16, isOutput=False)
    # raw h-table (transposed); w-table windowed: win[:, 64p+32j+r] = T[:, 2p+j+r]
    rh_tbl = nc.declare_dram_parameter("rh_tbl", [HD, 2 * H - 1], BF16,
                                       isOutput=False)
    rw_win = nc.declare_dram_parameter("rw_win", [HD, S], BF16, isOutput=False)
    onehot = nc.declare_dram_parameter("onehot", [64, S], BF16, isOutput=False)
    out_a = nc.declare_dram_parameter("out_a", [S, C], BF16, isOutput=True)
    out_b = nc.declare_dram_parameter("out_b", [S, C], BF16, isOutput=True)
    # DRAM bounce for the h-axis band gather (I/O tensors; internal DRAM
    # scratch is paged and slower for strided DMAs)
    ph_dram = nc.declare_dram_parameter("ph_dram", [(2 * H - 1) * S], BF16,
                                        isOutput=True)

    with tile.TileContext(nc) as tc:
        with (
            tc.tile_pool(name="persist", bufs=1) as persist,
            tc.tile_pool(name="ps_sc", bufs=2, space="PSUM") as ps_sc,
            tc.tile_pool(name="ps_av", bufs=1, space="PSUM") as ps_av,
            tc.tile_pool(name="ps_aux", bufs=1, space="PSUM") as ps_aux,
            tc.tile_pool(name="et", bufs=10) as et_pool,
            tc.tile_pool(name="small", bufs=2) as small,
        ):
            # ---- persistent SBUF tiles (chunk ci lives at a column block of
            # one big tile; one or two large DMAs fill each tile) ----
            xT_sb = persist.tile([128, NCH * S], BF16, tag="xT", name="xT_sb")
            wqk_sb = persist.tile([128, NCH * 2 * HPC * HD], BF16, tag="wqk",
                                  name="wqk_sb")
            wv_sb = persist.tile([128, NCH * HPC * HD], BF16, tag="wv",
                                 name="wv_sb")
            wproj_sb = persist.tile([128, 3 * C], BF16, tag="wproj",
                                    name="wproj_sb")

            def load_chunked(eng, dst, dram, ncol, c0, c1):
                # chunks [c0,c1) of DRAM [C, ncol] -> dst cols [ncol*c0 ...)
                src = bass.AP(dram, c0 * 128 * ncol,
                              [[ncol, 128], [128 * ncol, c1 - c0], [1, ncol]])
                d = bass.AP(dst.tensor, dst[:].offset + c0 * ncol,
                            [dst[:].ap[0], [ncol, c1 - c0], [1, ncol]])
                eng.dma_start(d, src)

            # sync queue: xT halves (compute-critical), then wv/wproj (late)
            load_chunked(nc.sync, xT_sb, xT, S, 0, 3)
            load_chunked(nc.sync, xT_sb, xT, S, 3, 6)
            # scalar queue (idle during lead-in): wqk halves + tables
            load_chunked(nc.scalar, wqk_sb, wqk, 2 * HPC * HD, 0, 3)
            load_chunked(nc.scalar, wqk_sb, wqk, 2 * HPC * HD, 3, 6)
            load_chunked(nc.sync, wv_sb, wv, HPC * HD, 0, 6)
            load_chunked(nc.sync, wproj_sb, wproj, C, 0, 3)
            rh_sb = persist.tile([HD, 2 * H - 1], BF16, tag="rh", name="rh_sb")
            nc.scalar.dma_start(rh_sb[:], rh_tbl[:, :])
            rw_sb = persist.tile([HD, S], BF16, tag="rw", name="rw_sb")
            nc.scalar.dma_start(rw_sb[:], rw_win[:, :])

            # ---- augmented k/q tiles (128, S) per head; one-hot rows DMAed
            # straight from DRAM into kaug rows 64-127 (scalar queue) ----
            kaug = [persist.tile([128, S], BF16, tag=f"kaug{i}", name=f"kaug{i}")
                    for i in range(HPC)]
            qaug = [persist.tile([128, S], BF16, tag=f"qaug{i}", name=f"qaug{i}")
                    for i in range(HPC)]
            for i in range(HPC):
                nc.scalar.dma_start(kaug[i][64:128, :], onehot[:, :])

            # ---- v tile: per-(k block, head) segment [ones (64) | v_i (64)]
            # (ones first so the av denominator lands on PSUM partitions 0-63:
            # reciprocal_approx_fast misreads PSUM at base_partition >= 64;
            # interleaved because the matmul stationary AP must be one
            # contiguous free dim) ----
            v_all = persist.tile([128, NKB * VSEG], BF16, tag="v", name="v_all")
            ones_dst = bass.AP(v_all.tensor, v_all[:].offset,
                               [v_all[:].ap[0], [2 * HD, NKB * HPC], [1, 64]])
            nc.gpsimd.memset(ones_dst, 1.0)

            # warm the exp activation table during the DMA lead-in
            warm = small.tile([1, 2], F32, tag="warm", name="warm")
            nc.gpsimd.memset(warm[:], 0.0)
            nc.scalar.activation(warm[:], warm[:],
                                 mybir.ActivationFunctionType.Exp)

            # ---- v projection (natural); emitted between head 0's
            # exp batch and its deferred avs so exps start earlier ----
            def v_proj_all():
                for sb in range(NKB):
                    vp = ps_aux.tile([128, S], F32, tag="aux", name="vp")
                    for ci in range(NCH):
                        nc.tensor.matmul(
                            vp[:, 0:HPC * HD],
                            xT_sb[:, S * ci + 128 * sb:S * ci + 128 * (sb + 1)],
                            wv_sb[:, HPC * HD * ci:HPC * HD * (ci + 1)],
                            start=(ci == 0), stop=(ci == NCH - 1))
                    vsrc = bass.AP(vp.tensor, vp[:].offset,
                                   [vp[:].ap[0], [HD, HPC], [1, HD]])
                    vdst = bass.AP(v_all.tensor,
                                   v_all[:].offset + VSEG * sb + HD,
                                   [v_all[:].ap[0], [2 * HD, HPC], [1, HD]])
                    nc.vector.tensor_copy(vdst, vsrc)

            # ---- qk projection (transposed layout) ----
            # qkT octile t covers oc rows [128t, 128t+128): t<3 -> q, t>=3 -> k
            def qk_octile(t, pool, tag):
                qp = pool.tile([128, S], F32, tag=tag, name="qp")
                for ci in range(NCH):
                    for nh in range(S // NHALF):
                        nc.tensor.matmul(
                            qp[:, NHALF * nh:NHALF * (nh + 1)],
                            wqk_sb[:, 768 * ci + 128 * t:768 * ci + 128 * (t + 1)],
                            xT_sb[:, S * ci + NHALF * nh:S * ci + NHALF * (nh + 1)],
                            start=(ci == 0), stop=(ci == NCH - 1))
                for sub in range(2):
                    head = (t % 3) * 2 + sub
                    dst = (qaug if t < 3 else kaug)[head]
                    cp = nc.scalar.copy if t in (0, 3) else nc.vector.tensor_copy
                    cp(dst[0:64, :], qp[64 * sub:64 * sub + 64, :])

            # ---- per head: direct banded rel-pos bias into qaug rows 64-127
            # BhT[r, (h,w)] = sum_c rhT[c, h+r] qT[c, (h,w)]  (r, h in [0,32))
            def band_extract(i):
                # heads 0-1 run in the lead-in where ScalarE is idle; the
                # rest overlap the exp-bound phase where VectorE has slack
                cp = nc.scalar.copy if i < 2 else nc.vector.tensor_copy
                # h-axis: PhT[j, q] = sum_c rhT[c, j] qT[c, q], then the band
                # shift qaug[64+r, (h,w)] = PhT[h+r, (h,w)] via one strided
                # DRAM-bounce DMA (engines stay free).
                ph = ps_aux.tile([2 * H - 1, S], F32, tag="aux", name="ph")
                for nh in range(S // NHALF):
                    sl = slice(NHALF * nh, NHALF * (nh + 1))
                    nc.tensor.matmul(ph[:, sl], rh_sb[:],
                                     qaug[i][0:64, sl], start=True, stop=True)
                ph_sb = small.tile([2 * H - 1, S], BF16, tag="ph_sb",
                                   name="ph_sb")
                cp(ph_sb[:], ph[:])
                nc.sync.dma_start(bass.AP(ph_dram, 0, [[S, 2 * H - 1], [1, S]]),
                                  ph_sb[:])
                nc.sync.dma_start(
                    bass.AP(qaug[i].tensor, qaug[i][:].offset + 64 * S,
                            [[S, 32], [W, H], [1, W]]),
                    bass.AP(ph_dram, 0, [[S, 32], [S + W, H], [1, W]]))
                bw = ps_aux.tile([64, S], F32, tag="aux", name="bw")
                pitch = bw[:].ap[0][0]
                for p in range(16):
                    rw = bass.AP(qaug[i].tensor, qaug[i][:].offset + 2 * p,
                                 [[S, 64], [1, 2], [W, H]])
                    nc.tensor.matmul(bw[:, 64 * p:64 * (p + 1)],
                                     rw_sb[:, 64 * p:64 * (p + 1)],
                                     rw, start=True, stop=True)
                # w-axis, j=0: w=2p / j=1: w=2p+1; dst col = 32h + w
                cp(bass.AP(qaug[i].tensor, qaug[i][:].offset + 96 * S,
                           [[S, 32], [2, 16], [W, H]]),
                   bass.AP(bw.tensor, bw[:].offset,
                           [[pitch, 32], [64, 16], [1, 32]]))
                cp(bass.AP(qaug[i].tensor, qaug[i][:].offset + 96 * S + 1,
                           [[S, 32], [2, 16], [W, H]]),
                   bass.AP(bw.tensor, bw[:].offset + 32 * pitch + 32,
                           [[pitch, 32], [64, 16], [1, 32]]))

            # ---- partial output projection for head chunks [c0, c1);
            # one qb tile per call, cast on DVE into a staging tile ----
            def proj_tile(qb, c0, c1, stage_sb):
                pp = ps_aux.tile([128, S], F32, tag="aux", name="pp")
                for ci in range(c0, c1):
                    st, sp = (ci == c0), (ci == c1 - 1)
                    nc.tensor.matmul(
                        pp[:, 0:NHALF],
                        out_headsT[:, S * ci + 128 * qb:S * ci + 128 * (qb + 1)],
                        wproj_sb[:, C * ci:C * ci + NHALF],
                        start=st, stop=sp)
                    nc.tensor.matmul(
                        pp[:, NHALF:C],
                        out_headsT[:, S * ci + 128 * qb:S * ci + 128 * (qb + 1)],
                        wproj_sb[:, C * ci + NHALF:C * (ci + 1)],
                        start=st, stop=sp)
                nc.vector.tensor_copy(stage_sb[:, C * qb:C * (qb + 1)],
                                      pp[:, 0:C])

            def proj_store(dram, stage_sb, q0, q1):
                src = bass.AP(stage_sb.tensor, stage_sb[:].offset + C * q0,
                              [stage_sb[:].ap[0], [C, q1 - q0], [1, C]])
                dst = bass.AP(dram, q0 * 128 * C,
                              [[C, 128], [128 * C, q1 - q0], [1, C]])
                nc.sync.dma_start(dst, src)

            # ---- attention per head; band extraction two heads ahead ----
            out_headsT = persist.tile([128, 3 * S], BF16, tag="ohT",
                                      name="ohT")
            pa_sb = persist.tile([128, NQB * C], BF16, tag="pa", name="pa_sb")
            pb_sb = persist.tile([128, NQB * C], BF16, tag="pb", name="pb_sb")

            # head 0/1's octiles ahead of the loop (octile 3 borrows a
            # sc-pool slot: the sc ring is idle during the lead-in); the
            # rest are emitted just before the first band that needs them,
            # so their PE work overlaps the exp-bound attention phase
            qk_octile(0, ps_aux, "aux")
            qk_octile(3, ps_sc, "sc")
            band_extract(0)
            band_extract(1)
            for i in range(HPC):
                if i + 2 < HPC:
                    if i % 2 == 0:
                        qk_octile((i + 2) // 2, ps_aux, "aux")
                        qk_octile((i + 2) // 2 + 3, ps_aux, "aux")
                    band_extract(i + 2)
                av = ps_av.tile([128, S], F32, tag="av", name="av")
                e_held = []
                for kb in range(NKB):
                    sc = ps_sc.tile([128, S], F32, tag="sc", name="sc")
                    for nh in range(S // NHALF):
                        sl = slice(NHALF * nh, NHALF * (nh + 1))
                        nc.tensor.matmul(
                            sc[:, sl],
                            kaug[i][:, 128 * kb:128 * (kb + 1)],
                            qaug[i][:, sl], start=True, stop=True)
                    e = et_pool.tile([128, S], BF16, tag="et", name="et")
                    nc.scalar.activation(e[:], sc[:],
                                         mybir.ActivationFunctionType.Exp)
                    if i == 0:
                        e_held.append(e)
                        continue
                    for nh in range(S // NHALF):
                        sl = slice(NHALF * nh, NHALF * (nh + 1))
                        nc.tensor.matmul(
                            av[:, sl],
                            v_all[:, VSEG * kb + 128 * i:VSEG * kb + 128 * (i + 1)],
                            e[:, sl],
                            start=(kb == 0), stop=(kb == NKB - 1))
                    if i == 4:
                        # chunks 0-1 of the output projection ride the PE
                        # slack inside head 4's exp-bound kb loop
                        proj_tile(kb, 0, 2, pa_sb)
                        if kb == NQB // 2 - 1:
                            proj_store(out_a, pa_sb, 0, NQB // 2)
                        elif kb == NQB - 1:
                            proj_store(out_a, pa_sb, NQB // 2, NQB)
                if i == 0:
                    v_proj_all()
                    for kb in range(NKB):
                        for nh in range(S // NHALF):
                            sl = slice(NHALF * nh, NHALF * (nh + 1))
                            nc.tensor.matmul(
                                av[:, sl],
                                v_all[:, VSEG * kb:VSEG * kb + 128],
                                e_held[kb][:, sl],
                                start=(kb == 0), stop=(kb == NKB - 1))
                # normalize: DVE drains PSUM (reciprocal + raw copy); the
                # multiply runs on the idle GpSimd (SBUF-only engine)
                rb = small.tile([64, S], F32, tag="rb", name="rb")
                nc.vector.reciprocal_approx_fast(rb[:], av[0:64, :])
                araw = small.tile([64, S], BF16, tag="araw", name="araw")
                nc.vector.tensor_copy(araw[:], av[64:128, :])
                chunk, row = i // 2, (i % 2) * 64
                nc.gpsimd.tensor_tensor(
                    out_headsT[row:row + 64, S * chunk:S * (chunk + 1)],
                    araw[:], rb[:], op=mybir.AluOpType.mult)

            # ---- tail: chunk-2 projection only (chunks 0-1 already stored) ----
            for qb in range(NQB):
                proj_tile(qb, 2, 3, pb_sb)
                if qb == NQB // 2 - 1:
                    proj_store(out_b, pb_sb, 0, NQB // 2)
                elif qb == NQB - 1:
                    proj_store(out_b, pb_sb, NQB // 2, NQB)

    nc.compile()
    return nc


def shard_inputs(x, Wqkv, Wproj, rel_pos_h, rel_pos_w):
    """Build the 8 per-core input maps."""
    import ml_dtypes
    bf16 = ml_dtypes.bfloat16
    scale = HD ** (-0.5)
    x = np.asarray(x, dtype=np.float32)
    Wqkv = np.asarray(Wqkv, dtype=np.float32)
    Wproj = np.asarray(Wproj, dtype=np.float32)
    rhT = np.ascontiguousarray(np.asarray(rel_pos_h, np.float32).T)
    rwT = np.ascontiguousarray(np.asarray(rel_pos_w, np.float32).T)

    def windowed(T):
        win = np.zeros((HD, S), np.float32)
        for p in range(16):
            for j in range(2):
                win[:, 64 * p + 32 * j:64 * p + 32 * j + 32] = \
                    T[:, 2 * p + j:2 * p + j + 32]
        return win.astype(bf16)

    rh_tbl = rhT.astype(bf16)
    rw_win = windowed(rwT)
    oh = np.zeros((64, S), np.float32)
    for khp in range(H):
        oh[khp, (31 - khp) * W:(31 - khp) * W + W] = 1.0
    for kwp in range(W):
        oh[32 + kwp, 31 - kwp::W] = 1.0
    oh = oh.astype(bf16)
    in_maps = []
    for core in range(NCORES):
        b = core // 2
        h0 = (core % 2) * HPC
        xb = x[b].reshape(S, C)
        xT = np.ascontiguousarray(xb.T).astype(bf16)
        wq = Wqkv[:, h0 * HD:(h0 + HPC) * HD]
        wk = Wqkv[:, C + h0 * HD:C + (h0 + HPC) * HD] * scale
        wqk = np.ascontiguousarray(np.concatenate([wq, wk], axis=1)).astype(bf16)
        wv = np.ascontiguousarray(
            Wqkv[:, 2 * C + h0 * HD:2 * C + (h0 + HPC) * HD]).astype(bf16)
        wp = np.ascontiguousarray(Wproj[h0 * HD:(h0 + HPC) * HD, :]).astype(bf16)
        in_maps.append({"xT": xT, "wqk": wqk, "wv": wv, "wproj": wp,
                        "rh_tbl": rh_tbl, "rw_win": rw_win, "onehot": oh})
    return in_maps


def gather_output(res, bproj):
    bproj = np.asarray(bproj, dtype=np.float32)
    out = np.empty((B, H, W, C), dtype=np.float32)
    for b in range(B):
        acc = bproj.copy()
        for core in (2 * b, 2 * b + 1):
            acc = acc + np.asarray(res.results[core]["out_a"], dtype=np.float32)
            acc = acc + np.asarray(res.results[core]["out_b"], dtype=np.float32)
        out[b] = acc.reshape(H, W, C)
    return out


_NC_CACHE = {}


def kernel(x, Wqkv, Wproj, bproj, rel_pos_h, rel_pos_w):
    if "nc" not in _NC_CACHE:
        _NC_CACHE["nc"] = build_program()
    nc = _NC_CACHE["nc"]
    in_maps = shard_inputs(x, Wqkv, Wproj, rel_pos_h, rel_pos_w)
    res = run_bass_kernel_spmd(nc, in_maps, list(range(NCORES)))
    return gather_output(res, bproj)


# revision 12
# speedup vs baseline: 1.0820x; 1.0820x over previous
"""Trainium2 Bass kernel for ViTDet-style attention with decomposed
relative-position bias.

Problem shapes (hardcoded):
  x: (4, 32, 32, 768) f32, Wqkv: (768, 2304), Wproj: (768, 768),
  bproj: (768,), rel_pos_h/w: (63, 64).
  12 heads, head_dim 64, S = 32*32 = 1024.

Sharding: 48 (batch, head) pairs -> 6 heads per core, all of one batch per
core-pair. Each core computes its heads' attention and TWO partial output
projections (chunks 0-1 of its heads' Wproj rows during head 4, chunk 2 at
the tail); the host sums the four partials per batch and adds bproj.

Device algorithm per core (bf16 matmuls, fp32 PSUM accumulation):
  - qkT = Wqk^T @ x^T  (x^T supplied pre-transposed by host; k pre-scaled).
    Inputs arrive as 12 chunk-granular DMAs split across the sync+scalar
    HWDGE queues so the first octile matmuls start as soon as chunk 0 lands.
  - v   = x @ Wv (natural layout), interleaved into head 0's kb loop.
  - rel-pos bias computed DIRECTLY in band form (no intermediate table
    product): BT[r, (h,w)] = sum_c rT[c, pos+r] qT[c, (h,w)].
    w-axis (all heads) and h-axis (heads 0-1, latency-critical): windowed
    stationaries, two positions packed per 64x64 matmul (diagonal blocks
    used) + strided scatter copies.
    h-axis (heads 2-5): table product + DRAM-bounce band-shift gather,
    split into two DMAs on separate queues (off-engine, pipelined 2 heads
    ahead).
  - scoresT (k x q) = kaugT^T @ qaugT in ONE K=128 matmul per tile:
    aug rows 0-63 = kT / qT, 64-95 = one-hot(h) / BhT, 96-127 = one-hot(w)/BwT
    => rel-pos bias folded into the QK matmul for free.
  - eT = exp(scoresT) on ScalarE (no max subtraction; scores are O(1)).
  - avT (128 x q): rows 0-63 = denominator (ones cols of v tile), rows
    64-127 = out accum. v tile layout [ones(64)|v_i(64)] per (kb, head)
    because the matmul stationary AP must be one contiguous free dim.
  - normalize: DVE reciprocal of av[0:64] + DVE multiply. No DMA bounce.
"""

import numpy as np

import concourse.bass as bass
import concourse.bacc as bacc
import concourse.mybir as mybir
import concourse.tile as tile
from concourse.bass_utils import run_bass_kernel_spmd

F32 = mybir.dt.float32
BF16 = mybir.dt.bfloat16

NH = 12          # total heads
C = 768
HD = 64
H = W = 32
S = H * W        # 1024
B = 4
NCORES = 8
HPC = NH * B // NCORES   # heads per core = 6
NCH = 6                  # C // 128 input-channel chunks
NKB = S // 128           # 8 k blocks
NQB = S // 128           # 8 q blocks
NHALF = 512              # matmul moving-dim half
VSEG = 2 * HD            # 128: [ones|v_i] per (k block, head)


def build_program():
    nc = bacc.Bacc("TRN2", target_bir_lowering=False, debug=False)

    xT = nc.declare_dram_parameter("xT", [C, S], BF16, isOutput=False)
    wqk = nc.declare_dram_parameter("wqk", [C, 2 * HPC * HD], BF16, isOutput=False)
    wv = nc.declare_dram_parameter("wv", [C, HPC * HD], BF16, isOutput=False)
    wproj = nc.declare_dram_parameter("wproj", [HPC * HD, C], BF16, isOutput=False)
    # raw h-table (transposed); windowed tables: win[:, 64p+32j+r] = T[:, 2p+j+r]
    rh_tbl = nc.declare_dram_parameter("rh_tbl", [HD, 2 * H - 1], BF16,
                                       isOutput=False)
    rh_win = nc.declare_dram_parameter("rh_win", [HD, S], BF16, isOutput=False)
    rw_win = nc.declare_dram_parameter("rw_win", [HD, S], BF16, isOutput=False)
    onehot = nc.declare_dram_parameter("onehot", [64, S], BF16, isOutput=False)
    out_a = nc.declare_dram_parameter("out_a", [S, C], BF16, isOutput=True)
    out_b = nc.declare_dram_parameter("out_b", [S, C], BF16, isOutput=True)
    # DRAM bounce for the h-axis band gather (I/O tensors; internal DRAM
    # scratch is paged and slower for strided DMAs)
    ph_dram = nc.declare_dram_parameter("ph_dram", [(2 * H - 1) * S], BF16,
                                        isOutput=True)

    with tile.TileContext(nc) as tc:
        with (
            tc.tile_pool(name="persist", bufs=1) as persist,
            tc.tile_pool(name="ps_sc", bufs=2, space="PSUM") as ps_sc,
            tc.tile_pool(name="ps_aux", bufs=2, space="PSUM") as ps_aux,
            tc.tile_pool(name="et", bufs=10) as et_pool,
            tc.tile_pool(name="small", bufs=2) as small,
        ):
            # ---- persistent SBUF tiles; chunk ci of each weight lives at a
            # column block. Loads are chunk-granular so compute starts as
            # soon as chunk 0 arrives, split across two HWDGE issue queues.
            xT_sb = persist.tile([128, NCH * S], BF16, tag="xT", name="xT_sb")
            wqk_sb = persist.tile([128, NCH * 2 * HPC * HD], BF16, tag="wqk",
                                  name="wqk_sb")
            wv_sb = persist.tile([128, NCH * HPC * HD], BF16, tag="wv",
                                 name="wv_sb")
            wproj_sb = persist.tile([128, 3 * C], BF16, tag="wproj",
                                    name="wproj_sb")

            def load_chunked(eng, dst, dram, ncol, c0, c1):
                # chunks [c0,c1) of DRAM [C, ncol] -> dst cols [ncol*c0 ...)
                src = bass.AP(dram, c0 * 128 * ncol,
                              [[ncol, 128], [128 * ncol, c1 - c0], [1, ncol]])
                d = bass.AP(dst.tensor, dst[:].offset + c0 * ncol,
                            [dst[:].ap[0], [ncol, c1 - c0], [1, ncol]])
                eng.dma_start(d, src)

            # interleave xT (sync) and wqk (scalar) per chunk; tables early
            # on scalar; wv/wproj later on sync (needed only mid-kernel)
            for ci in range(NCH):
                load_chunked(nc.sync, xT_sb, xT, S, ci, ci + 1)
                load_chunked(nc.scalar, wqk_sb, wqk, 2 * HPC * HD, ci, ci + 1)
            rh_sb = persist.tile([HD, 2 * H - 1], BF16, tag="rh", name="rh_sb")
            nc.scalar.dma_start(rh_sb[:], rh_tbl[:, :])
            rhw_sb = persist.tile([HD, S], BF16, tag="rhw", name="rhw_sb")
            nc.scalar.dma_start(rhw_sb[:], rh_win[:, :])
            rw_sb = persist.tile([HD, S], BF16, tag="rw", name="rw_sb")
            nc.scalar.dma_start(rw_sb[:], rw_win[:, :])
            load_chunked(nc.sync, wv_sb, wv, HPC * HD, 0, 3)
            load_chunked(nc.sync, wv_sb, wv, HPC * HD, 3, 6)
            load_chunked(nc.sync, wproj_sb, wproj, C, 0, 3)

            # ---- augmented k/q tiles (128, S) per head; one-hot rows DMAed
            # straight from DRAM into kaug rows 64-127 (scalar queue) ----
            kaug = [persist.tile([128, S], BF16, tag=f"kaug{i}", name=f"kaug{i}")
                    for i in range(HPC)]
            qaug = [persist.tile([128, S], BF16, tag=f"qaug{i}", name=f"qaug{i}")
                    for i in range(HPC)]
            for i in range(HPC):
                nc.scalar.dma_start(kaug[i][64:128, :], onehot[:, :])

            # ---- v tile: per-(k block, head) segment [ones (64) | v_i (64)]
            # (ones first so the av denominator lands on PSUM partitions 0-63:
            # reciprocal_approx_fast misreads PSUM at base_partition >= 64)
            v_all = persist.tile([128, NKB * HPC * VSEG], BF16, tag="v",
                                 name="v_all")
            ones_dst = bass.AP(v_all.tensor, v_all[:].offset,
                               [v_all[:].ap[0], [VSEG, NKB * HPC], [1, 64]])
            nc.gpsimd.memset(ones_dst, 1.0)

            def vslice(kb, i):
                off = HPC * VSEG * kb + VSEG * i
                return v_all[:, off:off + VSEG]

            # warm the exp activation table during the DMA lead-in
            warm = small.tile([1, 2], F32, tag="warm", name="warm")
            nc.gpsimd.memset(warm[:], 0.0)
            nc.scalar.activation(warm[:], warm[:],
                                 mybir.ActivationFunctionType.Exp)

            # ---- v projection for one k block (natural layout) + DVE cast
            # into the interleaved v tile. vp is a 1-bank PSUM tile so the
            # aux ring ping-pongs fill/drain across k blocks. ----
            def v_proj(sb):
                vp = ps_aux.tile([128, HPC * HD], F32, tag="aux", name="vp")
                for ci in range(NCH):
                    nc.tensor.matmul(
                        vp[:],
                        xT_sb[:, S * ci + 128 * sb:S * ci + 128 * (sb + 1)],
                        wv_sb[:, HPC * HD * ci:HPC * HD * (ci + 1)],
                        start=(ci == 0), stop=(ci == NCH - 1))
                vsrc = bass.AP(vp.tensor, vp[:].offset,
                               [vp[:].ap[0], [HD, HPC], [1, HD]])
                vdst = bass.AP(v_all.tensor,
                               v_all[:].offset + HPC * VSEG * sb + HD,
                               [v_all[:].ap[0], [VSEG, HPC], [1, HD]])
                nc.vector.tensor_copy(vdst, vsrc)

            # ---- qk projection (transposed layout) ----
            # qkT octile t covers oc rows [128t, 128t+128): t<3 -> q, t>=3 -> k
            # ci-major so each matmul starts as soon as its chunk arrives;
            # computed in two 512-col halves (1 PSUM bank each) so the aux
            # ring ping-pongs PE fill against the drain copies
            def qk_octile(t):
                for nh in range(S // NHALF):
                    qp = ps_aux.tile([128, NHALF], F32, tag="aux", name="qp")
                    for ci in range(NCH):
                        nc.tensor.matmul(
                            qp[:],
                            wqk_sb[:, 768 * ci + 128 * t:768 * ci + 128 * (t + 1)],
                            xT_sb[:, S * ci + NHALF * nh:S * ci + NHALF * (nh + 1)],
                            start=(ci == 0), stop=(ci == NCH - 1))
                    for sub in range(2):
                        head = (t % 3) * 2 + sub
                        dst = (qaug if t < 3 else kaug)[head]
                        cp = (nc.scalar.copy if t in (0, 3)
                              else nc.vector.tensor_copy)
                        cp(dst[0:64, NHALF * nh:NHALF * (nh + 1)],
                           qp[64 * sub:64 * sub + 64, :])

            # ---- banded rel-pos bias via windowed stationaries:
            # 16 matmuls, two positions packed per 64x64 (diagonal blocks
            # used, off-diagonal garbage ignored), then 2 scatter copies.
            # axis=0 -> h band into qaug rows 64-95 (moving cols h-paired)
            # axis=1 -> w band into qaug rows 96-127 (moving cols w-paired)
            def band_windowed(i, axis, cp):
                win = rhw_sb if axis == 0 else rw_sb
                rowbase = 64 if axis == 0 else 96
                for hh in range(2):       # window halves p in [8hh, 8hh+8)
                    bw = ps_aux.tile([64, NHALF], F32, tag="aux", name="bw")
                    pitch = bw[:].ap[0][0]
                    for pp_ in range(8):
                        p = 8 * hh + pp_
                        if axis == 0:
                            mv = bass.AP(qaug[i].tensor,
                                         qaug[i][:].offset + 2 * p * W,
                                         [[S, 64], [W, 2], [1, W]])
                        else:
                            mv = bass.AP(qaug[i].tensor,
                                         qaug[i][:].offset + 2 * p,
                                         [[S, 64], [1, 2], [W, H]])
                        nc.tensor.matmul(bw[:, 64 * pp_:64 * (pp_ + 1)],
                                         win[:, 64 * p:64 * (p + 1)],
                                         mv, start=True, stop=True)
                    for j in range(2):    # h/w = 2p+j; dst col = 32h + w
                        if axis == 0:
                            dst = bass.AP(
                                qaug[i].tensor,
                                qaug[i][:].offset + rowbase * S
                                + 2 * W * 8 * hh + j * W,
                                [[S, 32], [2 * W, 8], [1, W]])
                        else:
                            dst = bass.AP(
                                qaug[i].tensor,
                                qaug[i][:].offset + rowbase * S
                                + 16 * hh + j,
                                [[S, 32], [2, 8], [W, H]])
                        cp(dst,
                           bass.AP(bw.tensor,
                                   bw[:].offset + j * (32 * pitch + 32),
                                   [[pitch, 32], [64, 8], [1, 32]]))

            # h band via table product + DRAM-bounce gather (heads 2-5;
            # off-engine, pipelined two heads ahead of use)
            def band_h_bounce(i):
                ph_sb = small.tile([2 * H - 1, S], BF16, tag="ph_sb",
                                   name="ph_sb")
                for nh in range(S // NHALF):
                    sl = slice(NHALF * nh, NHALF * (nh + 1))
                    ph = ps_aux.tile([2 * H - 1, NHALF], F32, tag="aux",
                                     name="ph")
                    nc.tensor.matmul(ph[:], rh_sb[:],
                                     qaug[i][0:64, sl], start=True, stop=True)
                    nc.vector.tensor_copy(ph_sb[:, sl], ph[:])
                nc.sync.dma_start(bass.AP(ph_dram, 0, [[S, 2 * H - 1], [1, S]]),
                                  ph_sb[:])
                # band-shift gather split across both HWDGE queues
                for half, eng in ((0, nc.sync), (1, nc.scalar)):
                    eng.dma_start(
                        bass.AP(qaug[i].tensor,
                                qaug[i][:].offset + (64 + 16 * half) * S,
                                [[S, 16], [W, H], [1, W]]),
                        bass.AP(ph_dram, 16 * half * S,
                                [[S, 16], [S + W, H], [1, W]]))

            def band_extract(i):
                if i < 2:
                    # lead-in: fully on-chip, ScalarE is idle
                    band_windowed(i, 0, nc.scalar.copy)
                    band_windowed(i, 1, nc.scalar.copy)
                else:
                    band_h_bounce(i)
                    band_windowed(i, 1, nc.vector.tensor_copy)

            # ---- partial output projection for head chunks [c0, c1);
            # two 1-bank PSUM pieces (512 + 256 cols) per q block ----
            def proj_tile(qb, c0, c1, stage_sb, cp):
                for w0, w1 in ((0, NHALF), (NHALF, C)):
                    pp = ps_aux.tile([128, w1 - w0], F32, tag="aux", name="pp")
                    for ci in range(c0, c1):
                        nc.tensor.matmul(
                            pp[:],
                            out_headsT[:, S * ci + 128 * qb:S * ci + 128 * (qb + 1)],
                            wproj_sb[:, C * ci + w0:C * ci + w1],
                            start=(ci == c0), stop=(ci == c1 - 1))
                    cp(stage_sb[:, C * qb + w0:C * qb + w1], pp[:])

            def proj_store(dram, stage_sb, q0, q1):
                src = bass.AP(stage_sb.tensor, stage_sb[:].offset + C * q0,
                              [stage_sb[:].ap[0], [C, q1 - q0], [1, C]])
                dst = bass.AP(dram, q0 * 128 * C,
                              [[C, 128], [128 * C, q1 - q0], [1, C]])
                nc.sync.dma_start(dst, src)

            # ---- attention per head; band extraction two heads ahead ----
            out_headsT = persist.tile([128, 3 * S], BF16, tag="ohT",
                                      name="ohT")
            pa_sb = persist.tile([128, NQB * C], BF16, tag="pa", name="pa_sb")
            pb_sb = persist.tile([128, NQB * C], BF16, tag="pb", name="pb_sb")

            # only head 0/1's octiles ahead of the loop; the rest are emitted
            # just before the first band that needs them, so their PE work
            # overlaps the exp-bound attention phase
            qk_octile(0)
            qk_octile(3)
            band_extract(0)
            band_extract(1)
            for i in range(HPC):
                if i + 2 < HPC:
                    if i % 2 == 0:
                        qk_octile((i + 2) // 2)
                        qk_octile((i + 2) // 2 + 3)
                    band_extract(i + 2)
                av = ps_aux.tile([128, S], F32, tag="av", bufs=1, name="av")
                pend = []        # (kb, e) waiting for their av matmuls
                for kb in range(NKB):
                    sc = ps_sc.tile([128, S], F32, tag="sc", name="sc")
                    for nh in range(S // NHALF):
                        sl = slice(NHALF * nh, NHALF * (nh + 1))
                        nc.tensor.matmul(
                            sc[:, sl],
                            kaug[i][:, 128 * kb:128 * (kb + 1)],
                            qaug[i][:, sl], start=True, stop=True)
                    e = et_pool.tile([128, S], BF16, tag="et", name="et")
                    nc.scalar.activation(e[:], sc[:],
                                         mybir.ActivationFunctionType.Exp)
                    pend.append((kb, e))
                    if i == 0:
                        # v projection rides head 0's exp-bound kb loop;
                        # av matmuls lag one kb behind the v cast
                        v_proj(kb)
                        if kb == 0:
                            continue
                    pkb, pe = pend.pop(0)
                    for nh in range(S // NHALF):
                        sl = slice(NHALF * nh, NHALF * (nh + 1))
                        nc.tensor.matmul(
                            av[:, sl], vslice(pkb, i), pe[:, sl],
                            start=(pkb == 0), stop=(pkb == NKB - 1))
                    if i == 4:
                        # chunks 0-1 of the output projection ride the PE
                        # slack inside head 4's exp-bound kb loop
                        proj_tile(kb, 0, 2, pa_sb, nc.vector.tensor_copy)
                        if kb == NQB // 2 - 1:
                            proj_store(out_a, pa_sb, 0, NQB // 2)
                        elif kb == NQB - 1:
                            proj_store(out_a, pa_sb, NQB // 2, NQB)
                for pkb, pe in pend:
                    for nh in range(S // NHALF):
                        sl = slice(NHALF * nh, NHALF * (nh + 1))
                        nc.tensor.matmul(
                            av[:, sl], vslice(pkb, i), pe[:, sl],
                            start=(pkb == 0), stop=(pkb == NKB - 1))
                rb = small.tile([64, S], F32, tag="rb", name="rb")
                nc.vector.reciprocal_approx_fast(rb[:], av[0:64, :])
                chunk, row = i // 2, (i % 2) * 64
                nc.vector.tensor_tensor(
                    out_headsT[row:row + 64, S * chunk:S * (chunk + 1)],
                    av[64:128, :], rb[:], op=mybir.AluOpType.mult)

            # ---- tail: chunk-2 projection only; casts on the now-idle
            # ScalarE, grouped stores ----
            for qb in range(NQB):
                proj_tile(qb, 2, 3, pb_sb, nc.scalar.copy)
                if qb == NQB // 2 - 1:
                    proj_store(out_b, pb_sb, 0, NQB // 2)
                elif qb == NQB - 1:
                    proj_store(out_b, pb_sb, NQB // 2, NQB)

    nc.compile()
    return nc


def shard_inputs(x, Wqkv, Wproj, rel_pos_h, rel_pos_w):
    """Build the 8 per-core input maps."""
    import ml_dtypes
    bf16 = ml_dtypes.bfloat16
    scale = HD ** (-0.5)
    x = np.asarray(x, dtype=np.float32)
    Wqkv = np.asarray(Wqkv, dtype=np.float32)
    Wproj = np.asarray(Wproj, dtype=np.float32)
    rhT = np.ascontiguousarray(np.asarray(rel_pos_h, np.float32).T)
    rwT = np.ascontiguousarray(np.asarray(rel_pos_w, np.float32).T)

    def windowed(T):
        win = np.zeros((HD, S), np.float32)
        for p in range(16):
            for j in range(2):
                win[:, 64 * p + 32 * j:64 * p + 32 * j + 32] = \
                    T[:, 2 * p + j:2 * p + j + 32]
        return win.astype(bf16)

    rh_tbl = rhT.astype(bf16)
    rh_win = windowed(rhT)
    rw_win = windowed(rwT)
    oh = np.zeros((64, S), np.float32)
    for khp in range(H):
        oh[khp, (31 - khp) * W:(31 - khp) * W + W] = 1.0
    for kwp in range(W):
        oh[32 + kwp, 31 - kwp::W] = 1.0
    oh = oh.astype(bf16)
    in_maps = []
    for core in range(NCORES):
        b = core // 2
        h0 = (core % 2) * HPC
        xb = x[b].reshape(S, C)
        xT = np.ascontiguousarray(xb.T).astype(bf16)
        wq = Wqkv[:, h0 * HD:(h0 + HPC) * HD]
        wk = Wqkv[:, C + h0 * HD:C + (h0 + HPC) * HD] * scale
        wqk = np.ascontiguousarray(np.concatenate([wq, wk], axis=1)).astype(bf16)
        wv = np.ascontiguousarray(
            Wqkv[:, 2 * C + h0 * HD:2 * C + (h0 + HPC) * HD]).astype(bf16)
        wp = np.ascontiguousarray(Wproj[h0 * HD:(h0 + HPC) * HD, :]).astype(bf16)
        in_maps.append({"xT": xT, "wqk": wqk, "wv": wv, "wproj": wp,
                        "rh_tbl": rh_tbl, "rh_win": rh_win, "rw_win": rw_win,
                        "onehot": oh})
    return in_maps


def gather_output(res, bproj):
    bproj = np.asarray(bproj, dtype=np.float32)
    out = np.empty((B, H, W, C), dtype=np.float32)
    for b in range(B):
        acc = bproj.copy()
        for core in (2 * b, 2 * b + 1):
            acc = acc + np.asarray(res.results[core]["out_a"], dtype=np.float32)
            acc = acc + np.asarray(res.results[core]["out_b"], dtype=np.float32)
        out[b] = acc.reshape(H, W, C)
    return out


_NC_CACHE = {}


def kernel(x, Wqkv, Wproj, bproj, rel_pos_h, rel_pos_w):
    if "nc" not in _NC_CACHE:
        _NC_CACHE["nc"] = build_program()
    nc = _NC_CACHE["nc"]
    in_maps = shard_inputs(x, Wqkv, Wproj, rel_pos_h, rel_pos_w)
    res = run_bass_kernel_spmd(nc, in_maps, list(range(NCORES)))
    return gather_output(res, bproj)


# revision 16
# speedup vs baseline: 1.0932x; 1.0103x over previous
"""Trainium2 Bass kernel for ViTDet-style attention with decomposed
relative-position bias.

Problem shapes (hardcoded):
  x: (4, 32, 32, 768) f32, Wqkv: (768, 2304), Wproj: (768, 768),
  bproj: (768,), rel_pos_h/w: (63, 64).
  12 heads, head_dim 64, S = 32*32 = 1024.

Sharding: 48 (batch, head) pairs -> 6 heads per core, all of one batch per
core-pair. Each core computes its heads' attention and TWO partial output
projections (chunks 0-1 of its heads' Wproj rows during head 4, chunk 2 at
the tail); the host sums the four partials per batch and adds bproj.

Device algorithm per core (bf16 matmuls, fp32 PSUM accumulation):
  - qkT = Wqk^T @ x^T  (x^T supplied pre-transposed by host; k pre-scaled).
    Inputs arrive as 12 chunk-granular DMAs split across the sync+scalar
    HWDGE queues so the first octile matmuls start as soon as chunk 0 lands.
  - v   = x @ Wv (natural layout), interleaved into head 0's kb loop.
  - rel-pos bias computed DIRECTLY in band form (no intermediate table
    product): BT[r, (h,w)] = sum_c rT[c, pos+r] qT[c, (h,w)].
    w-axis (all heads) and h-axis (heads 0-1, latency-critical): windowed
    stationaries, two positions packed per 64x64 matmul (diagonal blocks
    used) + strided scatter copies.
    h-axis (heads 2-5): table product + DRAM-bounce band-shift gather,
    split into two DMAs on separate queues (off-engine, pipelined 2 heads
    ahead).
  - scoresT (k x q) = kaugT^T @ qaugT in ONE K=128 matmul per tile:
    aug rows 0-63 = kT / qT, 64-95 = one-hot(h) / BhT, 96-127 = one-hot(w)/BwT
    => rel-pos bias folded into the QK matmul for free.
  - eT = exp(scoresT) on ScalarE (no max subtraction; scores are O(1)).
  - avT (128 x q): rows 0-63 = denominator (ones cols of v tile), rows
    64-127 = out accum. v tile layout [ones(64)|v_i(64)] per (kb, head)
    because the matmul stationary AP must be one contiguous free dim.
  - normalize: DVE reciprocal of av[0:64] + DVE multiply. No DMA bounce.
"""

import numpy as np

import concourse.bass as bass
import concourse.bacc as bacc
import concourse.mybir as mybir
import concourse.tile as tile
from concourse.bass_utils import run_bass_kernel_spmd

F32 = mybir.dt.float32
BF16 = mybir.dt.bfloat16

NH = 12          # total heads
C = 768
HD = 64
H = W = 32
S = H * W        # 1024
B = 4
NCORES = 8
HPC = NH * B // NCORES   # heads per core = 6
NCH = 6                  # C // 128 input-channel chunks
NKB = S // 128           # 8 k blocks
NQB = S // 128           # 8 q blocks
NHALF = 512              # matmul moving-dim half
VSEG = 2 * HD            # 128: [ones|v_i] per (k block, head)


def build_program():
    nc = bacc.Bacc("TRN2", target_bir_lowering=False, debug=False)

    xT = nc.declare_dram_parameter("xT", [C, S], BF16, isOutput=False)
    wqk = nc.declare_dram_parameter("wqk", [C, 2 * HPC * HD], BF16, isOutput=False)
    wv = nc.declare_dram_parameter("wv", [C, HPC * HD], BF16, isOutput=False)
    wproj = nc.declare_dram_parameter("wproj", [HPC * HD, C], BF16, isOutput=False)
    # raw h-table (transposed); windowed tables: win[:, 64p+32j+r] = T[:, 2p+j+r]
    rh_tbl = nc.declare_dram_parameter("rh_tbl", [HD, 2 * H - 1], BF16,
                                       isOutput=False)
    rh_win = nc.declare_dram_parameter("rh_win", [HD, S], BF16, isOutput=False)
    rw_win = nc.declare_dram_parameter("rw_win", [HD, S], BF16, isOutput=False)
    onehot = nc.declare_dram_parameter("onehot", [64, S], BF16, isOutput=False)
    out_a = nc.declare_dram_parameter("out_a", [S, C], BF16, isOutput=True)
    out_b = nc.declare_dram_parameter("out_b", [S, C], BF16, isOutput=True)
    # DRAM bounce for the h-axis band gather (I/O tensors; internal DRAM
    # scratch is paged and slower for strided DMAs)
    ph_dram = nc.declare_dram_parameter("ph_dram", [(2 * H - 1) * S], BF16,
                                        isOutput=True)

    with tile.TileContext(nc) as tc:
        with (
            tc.tile_pool(name="persist", bufs=1) as persist,
            tc.tile_pool(name="ps_sc", bufs=2, space="PSUM") as ps_sc,
            tc.tile_pool(name="ps_aux", bufs=2, space="PSUM") as ps_aux,
            tc.tile_pool(name="et", bufs=10) as et_pool,
            tc.tile_pool(name="small", bufs=2) as small,
        ):
            # ---- persistent SBUF tiles; chunk ci of each weight lives at a
            # column block. Loads are chunk-granular so compute starts as
            # soon as chunk 0 arrives, split across two HWDGE issue queues.
            xT_sb = persist.tile([128, NCH * S], BF16, tag="xT", name="xT_sb")
            wqk_sb = persist.tile([128, NCH * 2 * HPC * HD], BF16, tag="wqk",
                                  name="wqk_sb")
            wv_sb = persist.tile([128, NCH * HPC * HD], BF16, tag="wv",
                                 name="wv_sb")
            wproj_sb = persist.tile([128, 3 * C], BF16, tag="wproj",
                                    name="wproj_sb")

            def load_chunked(eng, dst, dram, ncol, c0, c1):
                # chunks [c0,c1) of DRAM [C, ncol] -> dst cols [ncol*c0 ...)
                src = bass.AP(dram, c0 * 128 * ncol,
                              [[ncol, 128], [128 * ncol, c1 - c0], [1, ncol]])
                d = bass.AP(dst.tensor, dst[:].offset + c0 * ncol,
                            [dst[:].ap[0], [ncol, c1 - c0], [1, ncol]])
                eng.dma_start(d, src)

            # interleave xT (sync) and wqk (scalar) per chunk; tables early
            # on scalar; wv/wproj later on sync (needed only mid-kernel)
            for ci in range(NCH):
                load_chunked(nc.sync, xT_sb, xT, S, ci, ci + 1)
                load_chunked(nc.scalar, wqk_sb, wqk, 2 * HPC * HD, ci, ci + 1)
            rh_sb = persist.tile([HD, 2 * H - 1], BF16, tag="rh", name="rh_sb")
            nc.scalar.dma_start(rh_sb[:], rh_tbl[:, :])
            rhw_sb = persist.tile([HD, S], BF16, tag="rhw", name="rhw_sb")
            nc.scalar.dma_start(rhw_sb[:], rh_win[:, :])
            rw_sb = persist.tile([HD, S], BF16, tag="rw", name="rw_sb")
            nc.scalar.dma_start(rw_sb[:], rw_win[:, :])
            load_chunked(nc.sync, wv_sb, wv, HPC * HD, 0, 3)
            load_chunked(nc.sync, wv_sb, wv, HPC * HD, 3, 6)
            load_chunked(nc.sync, wproj_sb, wproj, C, 0, 3)

            # ---- augmented k/q tiles (128, S) per head; one-hot rows DMAed
            # straight from DRAM into kaug rows 64-127 (scalar queue) ----
            kaug = [persist.tile([128, S], BF16, tag=f"kaug{i}", name=f"kaug{i}")
                    for i in range(HPC)]
            qaug = [persist.tile([128, S], BF16, tag=f"qaug{i}", name=f"qaug{i}")
                    for i in range(HPC)]
            for i in range(HPC):
                nc.scalar.dma_start(kaug[i][64:128, :], onehot[:, :])

            # ---- v tile: per-(k block, head) segment [ones (64) | v_i (64)]
            # (ones first so the av denominator lands on PSUM partitions 0-63:
            # reciprocal_approx_fast misreads PSUM at base_partition >= 64)
            v_all = persist.tile([128, NKB * HPC * VSEG], BF16, tag="v",
                                 name="v_all")
            ones_dst = bass.AP(v_all.tensor, v_all[:].offset,
                               [v_all[:].ap[0], [VSEG, NKB * HPC], [1, 64]])
            nc.gpsimd.memset(ones_dst, 1.0)

            def vslice(kb, i):
                off = HPC * VSEG * kb + VSEG * i
                return v_all[:, off:off + VSEG]

            # warm the exp activation table during the DMA lead-in
            warm = small.tile([1, 2], F32, tag="warm", name="warm")
            nc.gpsimd.memset(warm[:], 0.0)
            nc.scalar.activation(warm[:], warm[:],
                                 mybir.ActivationFunctionType.Exp)

            # ---- v projection for one k block (natural layout) + DVE cast
            # into the interleaved v tile. vp is a 1-bank PSUM tile so the
            # aux ring ping-pongs fill/drain across k blocks. ----
            def v_proj(sb):
                vp = ps_aux.tile([128, HPC * HD], F32, tag="aux", name="vp")
                for ci in range(NCH):
                    nc.tensor.matmul(
                        vp[:],
                        xT_sb[:, S * ci + 128 * sb:S * ci + 128 * (sb + 1)],
                        wv_sb[:, HPC * HD * ci:HPC * HD * (ci + 1)],
                        start=(ci == 0), stop=(ci == NCH - 1))
                vsrc = bass.AP(vp.tensor, vp[:].offset,
                               [vp[:].ap[0], [HD, HPC], [1, HD]])
                vdst = bass.AP(v_all.tensor,
                               v_all[:].offset + HPC * VSEG * sb + HD,
                               [v_all[:].ap[0], [VSEG, HPC], [1, HD]])
                nc.vector.tensor_copy(vdst, vsrc)

            # ---- qk projection (transposed layout) ----
            # qkT octile t covers oc rows [128t, 128t+128): t<3 -> q, t>=3 -> k
            # ci-major so each matmul starts as soon as its chunk arrives;
            # computed in two 512-col halves (1 PSUM bank each) so the aux
            # ring ping-pongs PE fill against the drain copies
            def qk_octile_half(t, nh):
                qp = ps_aux.tile([128, NHALF], F32, tag="aux", name="qp")
                for ci in range(NCH):
                    nc.tensor.matmul(
                        qp[:],
                        wqk_sb[:, 768 * ci + 128 * t:768 * ci + 128 * (t + 1)],
                        xT_sb[:, S * ci + NHALF * nh:S * ci + NHALF * (nh + 1)],
                        start=(ci == 0), stop=(ci == NCH - 1))
                for sub in range(2):
                    head = (t % 3) * 2 + sub
                    dst = (qaug if t < 3 else kaug)[head]
                    cp = (nc.scalar.copy if t in (0, 3)
                          else nc.vector.tensor_copy)
                    cp(dst[0:64, NHALF * nh:NHALF * (nh + 1)],
                       qp[64 * sub:64 * sub + 64, :])

            def qk_octile(t):
                for nh in range(S // NHALF):
                    qk_octile_half(t, nh)

            # ---- banded rel-pos bias via windowed stationaries:
            # 16 matmuls, two positions packed per 64x64 (diagonal blocks
            # used, off-diagonal garbage ignored), then 2 scatter copies.
            # axis=0 -> h band into qaug rows 64-95 (moving cols h-paired)
            # axis=1 -> w band into qaug rows 96-127 (moving cols w-paired)
            def band_windowed(i, axis, cp):
                win = rhw_sb if axis == 0 else rw_sb
                rowbase = 64 if axis == 0 else 96
                for hh in range(2):       # window halves p in [8hh, 8hh+8)
                    bw = ps_aux.tile([64, NHALF], F32, tag="aux", name="bw")
                    pitch = bw[:].ap[0][0]
                    for pp_ in range(8):
                        p = 8 * hh + pp_
                        if axis == 0:
                            mv = bass.AP(qaug[i].tensor,
                                         qaug[i][:].offset + 2 * p * W,
                                         [[S, 64], [W, 2], [1, W]])
                        else:
                            mv = bass.AP(qaug[i].tensor,
                                         qaug[i][:].offset + 2 * p,
                                         [[S, 64], [1, 2], [W, H]])
                        nc.tensor.matmul(bw[:, 64 * pp_:64 * (pp_ + 1)],
                                         win[:, 64 * p:64 * (p + 1)],
                                         mv, start=True, stop=True)
                    for j in range(2):    # h/w = 2p+j; dst col = 32h + w
                        if axis == 0:
                            dst = bass.AP(
                                qaug[i].tensor,
                                qaug[i][:].offset + rowbase * S
                                + 2 * W * 8 * hh + j * W,
                                [[S, 32], [2 * W, 8], [1, W]])
                        else:
                            dst = bass.AP(
                                qaug[i].tensor,
                                qaug[i][:].offset + rowbase * S
                                + 16 * hh + j,
                                [[S, 32], [2, 8], [W, H]])
                        cp(dst,
                           bass.AP(bw.tensor,
                                   bw[:].offset + j * (32 * pitch + 32),
                                   [[pitch, 32], [64, 8], [1, 32]]))

            # h band via table product + DRAM-bounce gather (heads 2-5;
            # off-engine, pipelined two heads ahead of use)
            def band_h_bounce(i):
                cp = nc.scalar.copy if i < 2 else nc.vector.tensor_copy
                ph_sb = small.tile([2 * H - 1, S], BF16, tag="ph_sb",
                                   name="ph_sb")
                for nh in range(S // NHALF):
                    sl = slice(NHALF * nh, NHALF * (nh + 1))
                    ph = ps_aux.tile([2 * H - 1, NHALF], F32, tag="aux",
                                     name="ph")
                    nc.tensor.matmul(ph[:], rh_sb[:],
                                     qaug[i][0:64, sl], start=True, stop=True)
                    cp(ph_sb[:, sl], ph[:])
                nc.sync.dma_start(bass.AP(ph_dram, 0, [[S, 2 * H - 1], [1, S]]),
                                  ph_sb[:])
                # band-shift gather split in halves; scalar's queue helps
                # only pre-attention (keep it clean once exps stream)
                for half, eng in ((0, nc.sync),
                                  (1, nc.scalar if i < 2 else nc.sync)):
                    eng.dma_start(
                        bass.AP(qaug[i].tensor,
                                qaug[i][:].offset + (64 + 16 * half) * S,
                                [[S, 16], [W, H], [1, W]]),
                        bass.AP(ph_dram, 16 * half * S,
                                [[S, 16], [S + W, H], [1, W]]))

            def band_extract(i):
                cp = nc.scalar.copy if i < 2 else nc.vector.tensor_copy
                band_h_bounce(i)
                band_windowed(i, 1, cp)

            # ---- partial output projection for head chunks [c0, c1);
            # two 1-bank PSUM pieces (512 + 256 cols) per q block ----
            def proj_tile(qb, c0, c1, stage_sb, cp):
                for w0, w1 in ((0, NHALF), (NHALF, C)):
                    pp = ps_aux.tile([128, w1 - w0], F32, tag="aux", name="pp")
                    for ci in range(c0, c1):
                        nc.tensor.matmul(
                            pp[:],
                            out_headsT[:, S * ci + 128 * qb:S * ci + 128 * (qb + 1)],
                            wproj_sb[:, C * ci + w0:C * ci + w1],
                            start=(ci == c0), stop=(ci == c1 - 1))
                    cp(stage_sb[:, C * qb + w0:C * qb + w1], pp[:])

            def proj_store(dram, stage_sb, q0, q1):
                src = bass.AP(stage_sb.tensor, stage_sb[:].offset + C * q0,
                              [stage_sb[:].ap[0], [C, q1 - q0], [1, C]])
                dst = bass.AP(dram, q0 * 128 * C,
                              [[C, 128], [128 * C, q1 - q0], [1, C]])
                nc.sync.dma_start(dst, src)

            # ---- attention per head; band extraction two heads ahead ----
            out_headsT = persist.tile([128, 3 * S], BF16, tag="ohT",
                                      name="ohT")
            pa_sb = persist.tile([128, NQB * C], BF16, tag="pa", name="pa_sb")
            pb_sb = persist.tile([128, NQB * C], BF16, tag="pb", name="pb_sb")

            # lead-in PE (overlaps the input DMAs): octiles for heads 0-3's
            # q and heads 0-1's k, plus head 0/1/2's bias bands. Everything
            # else is queued as fillers consumed one per kb slot, so no big
            # PE block ever sits between a head's exp stream and the next.
            from collections import deque
            qk_octile(0)
            qk_octile(3)
            band_extract(0)
            band_extract(1)
            qk_octile(1)
            qk_octile(4)
            band_extract(2)
            fillers = deque()
            for t in (2, 5):
                for nh in range(S // NHALF):
                    fillers.append(lambda t=t, nh=nh: qk_octile_half(t, nh))
            for j in (3, 4, 5):
                fillers.append(lambda j=j: band_h_bounce(j))
                fillers.append(lambda j=j: band_windowed(
                    j, 1, nc.vector.tensor_copy))

            for i in range(HPC):
                av = ps_aux.tile([128, S], F32, tag="av", bufs=1, name="av")
                pend = []        # (kb, e) waiting for their av matmuls
                lag = 2 if i == 0 else 1

                def flush_av(i=i, pend=pend):
                    pkb, pe = pend.pop(0)
                    for nh in range(S // NHALF):
                        sl = slice(NHALF * nh, NHALF * (nh + 1))
                        nc.tensor.matmul(
                            av[:, sl], vslice(pkb, i), pe[:, sl],
                            start=(pkb == 0), stop=(pkb == NKB - 1))

                for kb in range(NKB):
                    sc = ps_sc.tile([128, S], F32, tag="sc", name="sc")
                    for nh in range(S // NHALF):
                        sl = slice(NHALF * nh, NHALF * (nh + 1))
                        nc.tensor.matmul(
                            sc[:, sl],
                            kaug[i][:, 128 * kb:128 * (kb + 1)],
                            qaug[i][:, sl], start=True, stop=True)
                    e = et_pool.tile([128, S], BF16, tag="et", name="et")
                    nc.scalar.activation(e[:], sc[:],
                                         mybir.ActivationFunctionType.Exp)
                    pend.append((kb, e))
                    if i == 0:
                        # v projection rides head 0's kb slots; avs lag
                        # two slots behind so each v cast has drained
                        v_proj(kb)
                    if len(pend) > lag:
                        flush_av()
                    if i != 0 and fillers:
                        fillers.popleft()()
                    if i == 4:
                        # chunks 0-1 of the output projection ride the PE
                        # slack inside head 4's exp-bound kb loop
                        proj_tile(kb, 0, 2, pa_sb, nc.vector.tensor_copy)
                        if kb == NQB // 2 - 1:
                            proj_store(out_a, pa_sb, 0, NQB // 2)
                        elif kb == NQB - 1:
                            proj_store(out_a, pa_sb, NQB // 2, NQB)
                while pend:
                    flush_av()
                rb = small.tile([64, S], F32, tag="rb", name="rb")
                nc.vector.reciprocal_approx_fast(rb[:], av[0:64, :])
                chunk, row = i // 2, (i % 2) * 64
                nc.vector.tensor_tensor(
                    out_headsT[row:row + 64, S * chunk:S * (chunk + 1)],
                    av[64:128, :], rb[:], op=mybir.AluOpType.mult)

            # ---- tail: chunk-2 projection only; casts on the now-idle
            # ScalarE, grouped stores ----
            for qb in range(NQB):
                proj_tile(qb, 2, 3, pb_sb, nc.scalar.copy)
                if qb == NQB // 2 - 1:
                    proj_store(out_b, pb_sb, 0, NQB // 2)
                elif qb == NQB - 1:
                    proj_store(out_b, pb_sb, NQB // 2, NQB)

    nc.compile()
    return nc


def shard_inputs(x, Wqkv, Wproj, rel_pos_h, rel_pos_w):
    """Build the 8 per-core input maps."""
    import ml_dtypes
    bf16 = ml_dtypes.bfloat16
    scale = HD ** (-0.5)
    x = np.asarray(x, dtype=np.float32)
    Wqkv = np.asarray(Wqkv, dtype=np.float32)
    Wproj = np.asarray(Wproj, dtype=np.float32)
    rhT = np.ascontiguousarray(np.asarray(rel_pos_h, np.float32).T)
    rwT = np.ascontiguousarray(np.asarray(rel_pos_w, np.float32).T)

    def windowed(T):
        win = np.zeros((HD, S), np.float32)
        for p in range(16):
            for j in range(2):
                win[:, 64 * p + 32 * j:64 * p + 32 * j + 32] = \
                    T[:, 2 * p + j:2 * p + j + 32]
        return win.astype(bf16)

    rh_tbl = rhT.astype(bf16)
    rh_win = windowed(rhT)
    rw_win = windowed(rwT)
    oh = np.zeros((64, S), np.float32)
    for khp in range(H):
        oh[khp, (31 - khp) * W:(31 - khp) * W + W] = 1.0
    for kwp in range(W):
        oh[32 + kwp, 31 - kwp::W] = 1.0
    oh = oh.astype(bf16)
    in_maps = []
    for core in range(NCORES):
        b = core // 2
        h0 = (core % 2) * HPC
        xb = x[b].reshape(S, C)
        xT = np.ascontiguousarray(xb.T).astype(bf16)
        wq = Wqkv[:, h0 * HD:(h0 + HPC) * HD]
        wk = Wqkv[:, C + h0 * HD:C + (h0 + HPC) * HD] * scale
        wqk = np.ascontiguousarray(np.concatenate([wq, wk], axis=1)).astype(bf16)
        wv = np.ascontiguousarray(
            Wqkv[:, 2 * C + h0 * HD:2 * C + (h0 + HPC) * HD]).astype(bf16)
        wp = np.ascontiguousarray(Wproj[h0 * HD:(h0 + HPC) * HD, :]).astype(bf16)
        in_maps.append({"xT": xT, "wqk": wqk, "wv": wv, "wproj": wp,
                        "rh_tbl": rh_tbl, "rh_win": rh_win, "rw_win": rw_win,
                        "onehot": oh})
    return in_maps


def gather_output(res, bproj):
    bproj = np.asarray(bproj, dtype=np.float32)
    out = np.empty((B, H, W, C), dtype=np.float32)
    for b in range(B):
        acc = bproj.copy()
        for core in (2 * b, 2 * b + 1):
            acc = acc + np.asarray(res.results[core]["out_a"], dtype=np.float32)
            acc = acc + np.asarray(res.results[core]["out_b"], dtype=np.float32)
        out[b] = acc.reshape(H, W, C)
    return out


_NC_CACHE = {}


def kernel(x, Wqkv, Wproj, bproj, rel_pos_h, rel_pos_w):
    if "nc" not in _NC_CACHE:
        _NC_CACHE["nc"] = build_program()
    nc = _NC_CACHE["nc"]
    in_maps = shard_inputs(x, Wqkv, Wproj, rel_pos_h, rel_pos_w)
    res = run_bass_kernel_spmd(nc, in_maps, list(range(NCORES)))
    return gather_output(res, bproj)
